# revision 2
# baseline (speedup 1.0000x reference)
"""Hybrid fp16 + fp8-DoubleRow split-K GEMM, v3.

v2 -> v3: f 0.24->0.256 (KO8=64), fp8 phase first in each block (smaller
boundary prefetch), deeper DMA prefetch (bufs 4/3), CK16=6.
"""

import numpy as np

P = 128
N_TOK = 2048
K = 32000
SV = 25000
N_CORES = 8
NPC = 3200
MB = 512
NB = 800
M_BLKS = N_TOK // MB   # 4
N_BLKS = NPC // NB     # 4
KO = K // P            # 250
KO8 = 64               # fp8 k-tiles (even)
KO16 = KO - KO8        # 186
CK16 = 6               # 31 chunks
CK8 = 16               # 4 chunks (8 pairs each)

_cache = {}


def _build(m_blks=M_BLKS, n_blks=N_BLKS, ko16=KO16, ko8=KO8):
    import concourse.bacc as bacc
    import concourse.mybir as mybir
    import concourse.tile as tile

    f16 = mybir.dt.float16
    f8 = mybir.dt.float8e4
    f32 = mybir.dt.float32
    DR = mybir.MatmulPerfMode.DoubleRow

    nc = bacc.Bacc(None, target_bir_lowering=False, debug=False)
    kxm = nc.dram_tensor("kxm", (P, m_blks, ko16, MB), f16, kind="ExternalInput")
    kxn = nc.dram_tensor("kxn", (P, n_blks, ko16, NB), f16, kind="ExternalInput")
    kxm8 = nc.dram_tensor("kxm8", (P, m_blks, ko8, MB), f8, kind="ExternalInput")
    kxn8 = nc.dram_tensor("kxn8", (P, n_blks, ko8, NB), f8, kind="ExternalInput")
    out = nc.dram_tensor("out", (P, m_blks * MB // P, n_blks * NB), f32,
                         kind="ExternalOutput")

    with tile.TileContext(nc) as tc:
        with tc.tile_pool(name="apool", bufs=4) as apool, \
             tc.tile_pool(name="bpool", bufs=4) as bpool, \
             tc.tile_pool(name="a8pool", bufs=3) as a8pool, \
             tc.tile_pool(name="b8pool", bufs=3) as b8pool, \
             tc.tile_pool(name="opool", bufs=4) as opool, \
             tc.tile_pool(name="pspool", bufs=1, space="PSUM") as pspool:
            for mb in range(m_blks):
                for nb in range(n_blks):
                    ps = [pspool.tile([P, NB], f32, name=f"ps{s}") for s in range(4)]
                    # --- fp8 DoubleRow phase (starts the accumulation) ---
                    n_chunks8 = ko8 // CK8
                    for kc in range(n_chunks8):
                        at8 = a8pool.tile([P, CK8, MB], f8, name="a8")
                        bt8 = b8pool.tile([P, CK8, NB], f8, name="b8")
                        nc.sync.dma_start(at8[:], kxm8[:, mb, kc * CK8:(kc + 1) * CK8, :])
                        nc.sync.dma_start(bt8[:], kxn8[:, nb, kc * CK8:(kc + 1) * CK8, :])
                        for t in range(CK8 // 2):
                            st = kc == 0 and t == 0
                            for ms in range(4):
                                lhsT = at8[:, 2 * t:2 * t + 2, ms * 128:(ms + 1) * 128]
                                nc.tensor.matmul(ps[ms][:, 0:512], lhsT,
                                                 bt8[:, 2 * t:2 * t + 2, 0:512],
                                                 start=st, stop=False, perf_mode=DR)
                                nc.tensor.matmul(ps[ms][:, 512:NB], lhsT,
                                                 bt8[:, 2 * t:2 * t + 2, 512:NB],
                                                 start=st, stop=False, perf_mode=DR)
                    # --- fp16 phase ---
                    n_chunks16 = ko16 // CK16
                    for kc in range(n_chunks16):
                        at = apool.tile([P, CK16, MB], f16, name="a")
                        bt = bpool.tile([P, CK16, NB], f16, name="b")
                        nc.sync.dma_start(at[:], kxm[:, mb, kc * CK16:(kc + 1) * CK16, :])
                        nc.sync.dma_start(bt[:], kxn[:, nb, kc * CK16:(kc + 1) * CK16, :])
                        for ki in range(CK16):
                            sp = kc == n_chunks16 - 1 and ki == CK16 - 1
                            for ms in range(4):
                                lhsT = at[:, ki, ms * 128:(ms + 1) * 128]
                                nc.tensor.matmul(ps[ms][:, 0:512], lhsT,
                                                 bt[:, ki, 0:512], start=False, stop=sp)
                                nc.tensor.matmul(ps[ms][:, 512:NB], lhsT,
                                                 bt[:, ki, 512:NB], start=False, stop=sp)
                    for ms in range(4):
                        ot = opool.tile([P, NB], f32, name="o")
                        nc.vector.tensor_copy(ot[:], ps[ms][:])
                        nc.sync.dma_start(
                            out[:, mb * 4 + ms, nb * NB:(nb + 1) * NB], ot[:])
    nc.compile()
    return nc


def _get_nc():
    if "nc" not in _cache:
        _cache["nc"] = _build()
    return _cache["nc"]


def _prep16(x, blks, bs, ko16):
    t = x[:, :ko16 * P].astype(np.float16)
    return np.ascontiguousarray(
        t.reshape(blks, bs, ko16, P).transpose(3, 0, 2, 1))


def _prep8(x, blks, bs, ko16, ko8):
    import ml_dtypes
    t = x[:, ko16 * P:(ko16 + ko8) * P].astype(ml_dtypes.float8_e4m3)
    return np.ascontiguousarray(
        t.reshape(blks, bs, ko8, P).transpose(3, 0, 2, 1))


def kernel(teacher_logits: np.ndarray, projection: np.ndarray) -> np.ndarray:
    from concourse.bass_utils import run_bass_kernel_spmd

    nc = _get_nc()

    teacher = np.asarray(teacher_logits, dtype=np.float32)
    proj = np.asarray(projection, dtype=np.float32)
    proj_pad = np.zeros((N_CORES * NPC, K), dtype=np.float32)
    proj_pad[:SV] = proj

    kxm_np = _prep16(teacher, M_BLKS, MB, KO16)
    kxm8_np = _prep8(teacher, M_BLKS, MB, KO16, KO8)

    in_maps = []
    for c in range(N_CORES):
        shard = proj_pad[c * NPC:(c + 1) * NPC]
        in_maps.append({
            "kxm": kxm_np,
            "kxm8": kxm8_np,
            "kxn": _prep16(shard, N_BLKS, NB, KO16),
            "kxn8": _prep8(shard, N_BLKS, NB, KO16, KO8),
        })

    res = run_bass_kernel_spmd(nc, in_maps, core_ids=list(range(N_CORES)))
    _cache["last_res"] = res

    parts = []
    for c in range(N_CORES):
        o = res.results[c]["out"]
        parts.append(o.transpose(1, 0, 2).reshape(N_TOK, NPC))
    full = np.concatenate(parts, axis=1)[:, :SV]
    return np.ascontiguousarray(full.astype(np.float32))


# revision 3
# speedup vs baseline: 1.2717x; 1.2717x over previous
"""Strassen(fp16) + fp8-DoubleRow hybrid GEMM for nn_LogitsProjector.

Per core: C[2048, 3200] = A[2048, 32000] @ B[3200, 32000].T
 - Last KO8*128 of K in fp8e4m3 DoubleRow (~2x rate), accumulated per
   output block and added into the C accumulators.
 - First KO16*128 of K in fp16 via one Strassen level (M, N, K all
   halved; 7 products instead of 8 -> 12.5% less tensor-engine work).
   Operand combos (A11+A22 etc.) are precomputed host-side in fp16.
 - C-quadrant combination runs on the vector engine into SBUF
   accumulators, overlapped with the next product's matmuls.
"""

import numpy as np

P = 128
N_TOK = 2048
K = 32000
SV = 25000
N_CORES = 8
NPC = 3200
MB = 512
NB = 800
KO = K // P            # 250
KO8 = 64               # fp8 k-tiles (even)
KO16 = KO - KO8        # 186 fp16 k-tiles; half = 93
CKS = 3                # strassen k-tiles per DMA chunk (divides KO16//2)
CK8 = 16               # fp8 k-tiles per chunk

# product -> (A-combo spec, B-combo spec) with blocks 1=11,2=12,3=21,4=22
# combos: (x, y, sign) meaning block_x + sign*block_y  (y=0 -> just x)
A_COMBOS = [(1, 4, 1), (3, 4, 1), (1, 0, 1), (4, 0, 1), (1, 2, 1),
            (3, 1, -1), (2, 4, -1)]
B_COMBOS = [(1, 4, 1), (1, 0, 1), (3, 4, -1), (2, 1, -1), (4, 0, 1),
            (1, 3, 1), (2, 4, 1)]
# product -> list of (quadrant, sign); quadrants 0=C11 1=C12 2=C21 3=C22
SIGNS = [[(0, 1), (3, 1)], [(2, 1), (3, -1)], [(1, 1), (3, 1)],
         [(0, 1), (2, 1)], [(0, -1), (1, 1)], [(3, 1)], [(0, 1)]]

_cache = {}


def _build(ko16=KO16, ko8=KO8, cks=CKS, ck8=CK8):
    import concourse.bacc as bacc
    import concourse.mybir as mybir
    import concourse.tile as tile

    f16 = mybir.dt.float16
    f8 = mybir.dt.float8e4
    f32 = mybir.dt.float32
    DR = mybir.MatmulPerfMode.DoubleRow
    ADD = mybir.AluOpType.add
    SUB = mybir.AluOpType.subtract

    k2 = ko16 // 2
    assert k2 % cks == 0 and ko8 % ck8 == 0 and ck8 % 2 == 0

    nc = bacc.Bacc(None, target_bir_lowering=False, debug=False)
    am = nc.dram_tensor("am", (P, 7, 2, k2, MB), f16, kind="ExternalInput")
    bm = nc.dram_tensor("bm", (P, 7, 2, k2, NB), f16, kind="ExternalInput")
    kxm8 = nc.dram_tensor("kxm8", (P, 4, ko8, MB), f8, kind="ExternalInput")
    kxn8 = nc.dram_tensor("kxn8", (P, 4, ko8, NB), f8, kind="ExternalInput")
    out = nc.dram_tensor("out", (P, N_TOK // P, NPC), f32, kind="ExternalOutput")

    with tile.TileContext(nc) as tc:
        with tc.tile_pool(name="apool", bufs=4) as apool, \
             tc.tile_pool(name="bpool", bufs=4) as bpool, \
             tc.tile_pool(name="a8pool", bufs=3) as a8pool, \
             tc.tile_pool(name="b8pool", bufs=3) as b8pool, \
             tc.tile_pool(name="cpool", bufs=1) as cpool, \
             tc.tile_pool(name="pspool", bufs=1, space="PSUM") as pspool:
            for i in range(2):
                for j in range(2):
                    c = [[cpool.tile([P, NB], f32, name=f"c{q}_{ms}")
                          for ms in range(4)] for q in range(4)]
                    touched = [False] * 4
                    for prod in range(7):
                        ps = [pspool.tile([P, NB], f32, name=f"ps{s}")
                              for s in range(4)]
                        for kc in range(k2 // cks):
                            at = apool.tile([P, cks, MB], f16, name="a")
                            bt = bpool.tile([P, cks, NB], f16, name="b")
                            nc.sync.dma_start(
                                at[:], am[:, prod, i, kc * cks:(kc + 1) * cks, :])
                            nc.sync.dma_start(
                                bt[:], bm[:, prod, j, kc * cks:(kc + 1) * cks, :])
                            for ki in range(cks):
                                kg = kc * cks + ki
                                st, sp = kg == 0, kg == k2 - 1
                                for ms in range(4):
                                    lhsT = at[:, ki, ms * 128:(ms + 1) * 128]
                                    nc.tensor.matmul(ps[ms][:, 0:512], lhsT,
                                                     bt[:, ki, 0:512],
                                                     start=st, stop=sp)
                                    nc.tensor.matmul(ps[ms][:, 512:NB], lhsT,
                                                     bt[:, ki, 512:NB],
                                                     start=st, stop=sp)
                        for (q, sgn) in SIGNS[prod]:
                            for ms in range(4):
                                if not touched[q]:
                                    nc.vector.tensor_copy(c[q][ms][:], ps[ms][:])
                                else:
                                    nc.vector.tensor_tensor(
                                        c[q][ms][:], c[q][ms][:], ps[ms][:],
                                        ADD if sgn > 0 else SUB)
                            touched[q] = True
                    # fp8 DoubleRow tail per quadrant block
                    for q in range(4):
                        mb = (q // 2) * 2 + i
                        nb = (q % 2) * 2 + j
                        ps = [pspool.tile([P, NB], f32, name=f"ps{s}")
                              for s in range(4)]
                        for kc in range(ko8 // ck8):
                            at8 = a8pool.tile([P, ck8, MB], f8, name="a8")
                            bt8 = b8pool.tile([P, ck8, NB], f8, name="b8")
                            nc.sync.dma_start(
                                at8[:], kxm8[:, mb, kc * ck8:(kc + 1) * ck8, :])
                            nc.sync.dma_start(
                                bt8[:], kxn8[:, nb, kc * ck8:(kc + 1) * ck8, :])
                            for t in range(ck8 // 2):
                                st = kc == 0 and t == 0
                                sp = kc == ko8 // ck8 - 1 and t == ck8 // 2 - 1
                                for ms in range(4):
                                    lhsT = at8[:, 2 * t:2 * t + 2,
                                               ms * 128:(ms + 1) * 128]
                                    nc.tensor.matmul(
                                        ps[ms][:, 0:512], lhsT,
                                        bt8[:, 2 * t:2 * t + 2, 0:512],
                                        start=st, stop=sp, perf_mode=DR)
                                    nc.tensor.matmul(
                                        ps[ms][:, 512:NB], lhsT,
                                        bt8[:, 2 * t:2 * t + 2, 512:NB],
                                        start=st, stop=sp, perf_mode=DR)
                        for ms in range(4):
                            nc.vector.tensor_tensor(
                                c[q][ms][:], c[q][ms][:], ps[ms][:], ADD)
                            nc.sync.dma_start(
                                out[:, mb * 4 + ms, nb * NB:(nb + 1) * NB],
                                c[q][ms][:])
    nc.compile()
    return nc


def _get_nc():
    if "nc" not in _cache:
        _cache["nc"] = _build()
    return _cache["nc"]


def _combo(blocks, spec):
    x, y, sgn = spec
    if y == 0:
        return blocks[x]
    return (blocks[x] + np.float16(sgn) * blocks[y]).astype(np.float16)


def _prep_strassen(x16, half_rows, k2p, combos):
    # x16: [2*half_rows, 2*k2p] fp16. blocks 1=11 2=12 3=21 4=22
    blocks = {1: x16[:half_rows, :k2p], 2: x16[:half_rows, k2p:],
              3: x16[half_rows:, :k2p], 4: x16[half_rows:, k2p:]}
    bs = half_rows // 2  # rows per sub-block (i index)
    k2 = k2p // P
    panels = []
    for spec in combos:
        cb = _combo(blocks, spec)  # [half_rows, k2p] fp16
        panels.append(cb.reshape(2, bs, k2, P).transpose(3, 0, 2, 1))
    return np.ascontiguousarray(np.stack(panels, axis=1))  # (P,7,2,k2,bs)


def _prep8(x, blks, bs, ko16, ko8):
    import ml_dtypes
    t = x[:, ko16 * P:(ko16 + ko8) * P].astype(ml_dtypes.float8_e4m3)
    return np.ascontiguousarray(
        t.reshape(blks, bs, ko8, P).transpose(3, 0, 2, 1))


def kernel(teacher_logits: np.ndarray, projection: np.ndarray) -> np.ndarray:
    from concourse.bass_utils import run_bass_kernel_spmd

    nc = _get_nc()

    teacher = np.asarray(teacher_logits, dtype=np.float32)
    proj = np.asarray(projection, dtype=np.float32)
    proj_pad = np.zeros((N_CORES * NPC, K), dtype=np.float32)
    proj_pad[:SV] = proj

    k16p = KO16 * P
    a16 = teacher[:, :k16p].astype(np.float16)
    am_np = _prep_strassen(a16, N_TOK // 2, k16p // 2, A_COMBOS)
    kxm8_np = _prep8(teacher, 4, MB, KO16, KO8)

    in_maps = []
    for c in range(N_CORES):
        shard = proj_pad[c * NPC:(c + 1) * NPC]
        b16 = shard[:, :k16p].astype(np.float16)
        in_maps.append({
            "am": am_np,
            "bm": _prep_strassen(b16, NPC // 2, k16p // 2, B_COMBOS),
            "kxm8": kxm8_np,
            "kxn8": _prep8(shard, 4, NB, KO16, KO8),
        })

    res = run_bass_kernel_spmd(nc, in_maps, core_ids=list(range(N_CORES)))
    _cache["last_res"] = res

    parts = []
    for c in range(N_CORES):
        o = res.results[c]["out"]
        parts.append(o.transpose(1, 0, 2).reshape(N_TOK, NPC))
    full = np.concatenate(parts, axis=1)[:, :SV]
    return np.ascontiguousarray(full.astype(np.float32))
